# revision 10
# baseline (speedup 1.0000x reference)
"""Trainium2 Bass kernel for masked tanh-clipped attention softmax.

Reference computation (B=16, NQ=NK=2048, KD=QD=KQ=256, CLIP=10):
    k = k_inputs @ Wk                     [B, NK, 256]
    q = q_inputs @ Wq                     [B, NQ, 256]
    s = (q @ k^T) / 16                    [B, NQ, NK]
    s = tanh(s) * 10
    s = where(adjancy, s, -inf)
    out = softmax(s, axis=2)

Kernel strategy (per NeuronCore, 2 batches each across 8 cores):
  * Fold the projections: A = Wq @ Wk^T (256x256), so s = q_in @ A @ k_in^T
    (fp16 matmul; A rounded to fp16).
  * Host passes q_in/k_in pre-transposed to [d, token] fp16, adjacency as
    uint8 scaled to {0,2}.
  * qaT = A^T @ q_inT; first chunk upfront, the rest threaded one chunk per
    m-tile through the PE's slack so the ACT pipeline never stalls.
  * Per m-tile (128 rows), software-pipelined at PAIR granularity:
      ACT: t = tanh(s/16)         (PSUM -> SBUF fp16, per tile; tile 0 is
                                   further split into 512-col chunks so ACT
                                   starts ~3us in)
      DVE: t += mask16            (mask16 = Pool-converted {0,2} fp16)
      ACT: e = exp(10t - 20)      (per PAIR: one 4096-wide op; masked
                                   entries <= e^-10, negligible)
      DVE: rowsum via tensor_scalar accum_out (4x mode), one reciprocal
           per pair, e *= 1/rowsum
  * ACT is the bottleneck engine (two table passes over 8.4M elems at
    0.833 ns/elem ~ 110us); everything else hides under it.
  * No PE warmup: the first tile's matmuls ramp the p-state while the
    startup DMAs stream (adjacency pair 0 first, then A, qT chunk 0, kT in
    512-col chunks so the first matmul starts after ~2us).
  * Output DMAs are dispatched from the Pool sequencer so an out-DMA
    waiting on its data never blocks SP-issued input prefetches.
  * Output fp16 to HBM; host upcasts to f32.
"""
import numpy as np

import concourse.bacc as bacc
import concourse.mybir as mybir
from concourse.tile import TileContext
from concourse.bass_utils import run_bass_kernel_spmd

F32 = mybir.dt.float32
F16 = mybir.dt.float16
U8 = mybir.dt.uint8
AF = mybir.ActivationFunctionType
ALU = mybir.AluOpType

B, NQ, NK = 16, 2048, 2048
D = 256                 # KD = QD = KQ
CORES = 8
BPC = B // CORES        # batches per core
MT = 128                # query rows per tile
NMT = NQ // MT          # 16 m-tiles per batch
CH = 512                # psum bank free-dim (fp32)
NCH = NK // CH          # 4 n-chunks per scores row


def build(reps=1):
    nc = bacc.Bacc(None, target_bir_lowering=False)

    qT = nc.dram_tensor("qT", [BPC, D, NQ], F16, kind="ExternalInput")
    kT = nc.dram_tensor("kT", [BPC, D, NK], F16, kind="ExternalInput")
    adj = nc.dram_tensor("adj", [BPC, NQ, NK], U8, kind="ExternalInput")
    # A = Wq @ Wk^T, folded on host (weight preprocessing), laid out as
    # a_in[p, dc, e] = A[dc*128 + p, e]
    a_in = nc.dram_tensor("a_in", [D, D], F16, kind="ExternalInput")
    out = nc.dram_tensor("out", [BPC, NQ, NK], F16, kind="ExternalOutput")

    with TileContext(nc) as tc:
        with (
            tc.tile_pool(name="const", bufs=1) as cp,
            tc.tile_pool(name="mt", bufs=4) as mp,
            tc.tile_pool(name="pair", bufs=3) as pp,
            tc.tile_pool(name="ps", bufs=2, space="PSUM") as ps,
        ):
            batches = sorted(set(bb for _ in range(reps) for bb in range(BPC)))
            b0 = batches[0]

            tiles = [(b, mt) for _ in range(reps) for b in batches
                     for mt in range(NMT)]
            npairs = len(tiles) // 2

            # ---- pair prefetch: adjacency DMA + Pool uint8->fp16 convert ----
            pair_m16 = {}

            def prefetch_pair(p):
                pb, pmt = tiles[2 * p]
                adj_t = pp.tile([128, 2, NK], U8, name="adj_t")
                nc.sync.dma_start(
                    out=adj_t[:],
                    in_=adj[pb, pmt * MT:(pmt + 2) * MT, :].rearrange(
                        "(t p) n -> p t n", p=128))
                m16 = pp.tile([128, 2, NK], F16, name="m16")
                nc.gpsimd.tensor_copy(m16[:], adj_t[:])
                pair_m16[p] = m16

            # ---- startup: order DMAs by when the pipeline needs them ----
            # Dispatch cost dominates the startup (each DMA holds HWDGE
            # ~650ns), so the first few loads go out on THREE engine queues
            # in parallel: A on DVE, qT chunk 0 on ACT (idle until the first
            # tanh), kT chunks + adjacency on SP. kT streams in 512-col
            # chunks so the first score matmul starts after ~2us.
            a_t = cp.tile([128, 2, D], F16)    # a_t[p, dc, e] = A[dc*128+p, e]
            nc.gpsimd.dma_start(out=a_t[:], in_=a_in.rearrange("(c p) e -> p c e", p=128))
            qT_ts, kT_ts, qa_ts = {}, {}, {}
            for b in batches:
                qT_ts[b] = cp.tile([128, 2, NQ], F16, name=f"qT{b}")
                kT_ts[b] = cp.tile([128, 2, NK], F16, name=f"kT{b}")
                qa_ts[b] = cp.tile([128, 2, NQ], F16, name=f"qa{b}")
            nc.scalar.dma_start(
                out=qT_ts[b0][:, :, 0:CH],
                in_=qT[b0, :, 0:CH].rearrange("(c p) m -> p c m", p=128))
            for n in range(2):
                nc.sync.dma_start(
                    out=kT_ts[b0][:, :, n * CH:(n + 1) * CH],
                    in_=kT[b0, :, n * CH:(n + 1) * CH].rearrange(
                        "(c p) m -> p c m", p=128))
            prefetch_pair(0)
            for n in range(2, NCH):
                nc.sync.dma_start(
                    out=kT_ts[b0][:, :, n * CH:(n + 1) * CH],
                    in_=kT[b0, :, n * CH:(n + 1) * CH].rearrange(
                        "(c p) m -> p c m", p=128))
            prefetch_pair(1)

            ebias = cp.tile([128, 1], F32)
            nc.vector.memset(ebias[:], -20.0)

            def late_loads(i):
                if i == 1:
                    for dc in range(2):
                        nc.sync.dma_start(out=qT_ts[b0][:, dc, CH:],
                                          in_=qT[b0, dc * 128:(dc + 1) * 128, CH:])
                elif i in (3, 5, 7, 9) and len(batches) > 1:
                    # halves, so adjacency prefetches interleave between them
                    b1 = batches[1]
                    j = (i - 3) // 2
                    dst, src = ((qT_ts[b1], qT) if j < 2 else (kT_ts[b1], kT))
                    h = j % 2
                    nc.sync.dma_start(
                        out=dst[:, :, h * NK // 2:(h + 1) * NK // 2],
                        in_=src[b1, :, h * NK // 2:(h + 1) * NK // 2].rearrange(
                            "(c p) m -> p c m", p=128))

            def qa_chunk(b, dp, mc):
                qa_ps = ps.tile([128, CH], F32, tag="sc", name="qa_ps")
                for dc in range(2):
                    nc.tensor.matmul(
                        qa_ps[:],
                        a_t[:, dc, dp * 128:(dp + 1) * 128],
                        qT_ts[b][:, dc, mc * CH:(mc + 1) * CH],
                        start=(dc == 0),
                        stop=(dc == 1),
                    )
                nc.vector.tensor_copy(
                    qa_ts[b][:, dp, mc * CH:(mc + 1) * CH], qa_ps[:])

            qa_jobs = [(b, dp, mc) for b in batches for mc in range(NCH)
                       for dp in range(2)]
            qa_chunk(*qa_jobs[0])
            qa_chunk(*qa_jobs[1])
            qa_left = qa_jobs[2:]

            # ---- software-pipelined m-tile loop, pair-granular epilogue ----
            # ACT order: tanh(2p), exp(pair p-1), tanh(2p+1), ... so the pair
            # exp (one 4096-wide op) fills ACT while DVE masks tile 2p and
            # the pair p-1 epilogue (rowsum/normalize) runs.
            pair_t = {}       # p -> tanh pair tile
            pair_e = {}       # p -> exp pair tile

            def pair_epilogue(p, split_dma=False):
                """rowsum + normalize + out DMA for pair p (deps all ready)."""
                pb, pmt = tiles[2 * p]
                t_pr = pair_t.pop(p)
                e_pr = pair_e.pop(p)
                rsum = mp.tile([128, 2], F32, bufs=2, name="rsum")
                rcp = mp.tile([128, 2], F32, bufs=2, name="rcp")
                for h in range(2):
                    nc.vector.tensor_scalar(
                        t_pr[:, h], e_pr[:, h], 1.0, 0.0,
                        op0=ALU.mult, op1=ALU.add, accum_out=rsum[:, h:h + 1])
                nc.vector.reciprocal(rcp[:], rsum[:])
                for h in range(2):
                    nc.vector.tensor_scalar_mul(e_pr[:, h], e_pr[:, h],
                                                rcp[:, h:h + 1])
                if split_dma:
                    for h in range(2):
                        nc.sync.dma_start(
                            out=out[pb, (pmt + h) * MT:(pmt + h + 1) * MT, :],
                            in_=e_pr[:, h])
                else:
                    nc.sync.dma_start(
                        out=out[pb, pmt * MT:(pmt + 2) * MT, :].rearrange(
                            "(t p) n -> p t n", p=128),
                        in_=e_pr[:])

            for idx, (b, mt) in enumerate(tiles):
                p, half = divmod(idx, 2)
                if half == 0:
                    pair_t[p] = mp.tile([128, 2, NK], F16, name="t_pr")
                # scores matmuls, n-outer so tile 0 only waits on kT chunk n
                sc_ps = ps.tile([128, NK], F32, tag="sc", name="sc_ps")
                for n in range(NCH):
                    for dp in range(2):
                        nc.tensor.matmul(
                            sc_ps[:, n * CH:(n + 1) * CH],
                            qa_ts[b][:, dp, mt * MT:(mt + 1) * MT],
                            kT_ts[b][:, dp, n * CH:(n + 1) * CH],
                            start=(dp == 0),
                            stop=(dp == 1),
                        )
                if qa_left and idx >= 1:
                    qa_chunk(*qa_left.pop(0))
                # tanh: tile 0 in 512-col chunks (starts as soon as psum
                # chunk 0 is done), the rest as one 2048-col op
                if idx == 0:
                    for n in range(NCH):
                        nc.scalar.activation(
                            pair_t[p][:, half, n * CH:(n + 1) * CH],
                            sc_ps[:, n * CH:(n + 1) * CH],
                            AF.Tanh, scale=1.0 / 16.0)
                else:
                    nc.scalar.activation(pair_t[p][:, half], sc_ps[:],
                                         AF.Tanh, scale=1.0 / 16.0)
                if half == 0 and p > 0:
                    # previous pair: one 4096-wide exp keeps ACT busy through
                    # this tile's DVE mask-add
                    e_pr = pp.tile([128, 2, NK], F16, name="e_pr")
                    nc.scalar.activation(e_pr[:], pair_t[p - 1][:], AF.Exp,
                                         scale=10.0, bias=ebias[:])
                    pair_e[p - 1] = e_pr
                # mask add (tensor_tensor runs in the 2x DVE mode)
                nc.vector.tensor_tensor(
                    pair_t[p][:, half], pair_t[p][:, half],
                    pair_m16[p][:, half], op=ALU.add)
                if half == 0 and p > 0:
                    pair_epilogue(p - 1)
                if idx == len(tiles) - 1:
                    # drain: last pair per tile, final tile in 512-col chunks
                    # so the tail is ~2us instead of a whole pair's epilogue
                    e_pr = pp.tile([128, 2, NK], F16, name="e_pr")
                    pair_e[p] = e_pr
                    t_pr = pair_t[p]
                    rsum = mp.tile([128, 2], F32, bufs=2, name="rsum")
                    rcp = mp.tile([128, 2], F32, bufs=2, name="rcp")
                    rcs = mp.tile([128, NCH], F32, bufs=2, name="rcs")
                    # tile 30: one-shot epilogue (overlaps tanh/exp of 31)
                    nc.scalar.activation(e_pr[:, 0], t_pr[:, 0], AF.Exp,
                                         scale=10.0, bias=ebias[:])
                    nc.vector.tensor_scalar(
                        t_pr[:, 0], e_pr[:, 0], 1.0, 0.0,
                        op0=ALU.mult, op1=ALU.add, accum_out=rsum[:, 0:1])
                    nc.vector.reciprocal(rcp[:, 0:1], rsum[:, 0:1])
                    nc.vector.tensor_scalar_mul(e_pr[:, 0], e_pr[:, 0],
                                                rcp[:, 0:1])
                    nc.sync.dma_start(
                        out=out[b, (mt - 1) * MT:mt * MT, :], in_=e_pr[:, 0])
                    # tile 31: chunked exp + rowsum, then chunked norm + DMA
                    for c in range(NCH):
                        sl = slice(c * CH, (c + 1) * CH)
                        nc.scalar.activation(e_pr[:, 1, sl], t_pr[:, 1, sl],
                                             AF.Exp, scale=10.0, bias=ebias[:])
                        nc.vector.tensor_scalar(
                            t_pr[:, 1, sl], e_pr[:, 1, sl], 1.0, 0.0,
                            op0=ALU.mult, op1=ALU.add,
                            accum_out=rcs[:, c:c + 1])
                    nc.vector.tensor_reduce(rsum[:, 1:2], rcs[:],
                                            op=ALU.add,
                                            axis=mybir.AxisListType.X)
                    nc.vector.reciprocal(rcp[:, 1:2], rsum[:, 1:2])
                    for c in range(NCH):
                        sl = slice(c * CH, (c + 1) * CH)
                        nc.vector.tensor_scalar_mul(e_pr[:, 1, sl],
                                                    e_pr[:, 1, sl],
                                                    rcp[:, 1:2])
                        nc.sync.dma_start(
                            out=out[b, mt * MT:(mt + 1) * MT, sl],
                            in_=e_pr[:, 1, sl])
                    pair_t.pop(p)
                    pair_e.pop(p)
                if half == 1:
                    late_loads(idx)
                    if p + 2 < npairs:
                        prefetch_pair(p + 2)
    nc.compile()
    return nc


_NC = None


def _get_nc():
    global _NC
    if _NC is None:
        _NC = build()
    return _NC


def kernel(k_inputs, q_inputs, adjancy, Wk, Wq):
    k_inputs = np.asarray(k_inputs, dtype=np.float32)
    q_inputs = np.asarray(q_inputs, dtype=np.float32)
    adjancy = np.asarray(adjancy, dtype=np.int32)
    Wk = np.asarray(Wk, dtype=np.float32)
    Wq = np.asarray(Wq, dtype=np.float32)
    nc = _get_nc()
    a_in = (Wq @ Wk.T).astype(np.float16)
    in_maps = []
    for c in range(CORES):
        lo, hi = c * BPC, (c + 1) * BPC
        in_maps.append({
            "qT": np.ascontiguousarray(
                q_inputs[lo:hi].transpose(0, 2, 1)).astype(np.float16),
            "kT": np.ascontiguousarray(
                k_inputs[lo:hi].transpose(0, 2, 1)).astype(np.float16),
            "adj": (adjancy[lo:hi] * 2).astype(np.uint8),
            "a_in": a_in,
        })
    res = run_bass_kernel_spmd(nc, in_maps, core_ids=list(range(CORES)))
    return np.concatenate(
        [res.results[c]["out"] for c in range(CORES)], axis=0
    ).astype(np.float32)


# revision 14
# speedup vs baseline: 1.0085x; 1.0085x over previous
"""Trainium2 Bass kernel for masked tanh-clipped attention softmax.

Reference computation (B=16, NQ=NK=2048, KD=QD=KQ=256, CLIP=10):
    k = k_inputs @ Wk                     [B, NK, 256]
    q = q_inputs @ Wq                     [B, NQ, 256]
    s = (q @ k^T) / 16                    [B, NQ, NK]
    s = tanh(s) * 10
    s = where(adjancy, s, -inf)
    out = softmax(s, axis=2)

Kernel strategy (per NeuronCore, 2 batches each across 8 cores):
  * Fold the projections: A = Wq @ Wk^T (256x256), so s = q_in @ A @ k_in^T
    (fp16 matmul; A rounded to fp16).
  * Host passes q_in/k_in pre-transposed to [d, token] fp16, adjacency as
    uint8 scaled to {0,2}.
  * qaT = A^T @ q_inT; first chunk upfront, the rest threaded one chunk per
    m-tile through the PE's slack so the ACT pipeline never stalls.
  * Per m-tile (128 rows), software-pipelined at PAIR granularity:
      ACT: t = tanh(s/16)         (PSUM -> SBUF fp16, per tile; tile 0 is
                                   further split into 512-col chunks so ACT
                                   starts ~3us in)
      DVE: t += mask16            (mask16 = Pool-converted {0,2} fp16)
      ACT: e = exp(10t - 20)      (per PAIR: one 4096-wide op; masked
                                   entries <= e^-10, negligible)
      DVE: rowsum via tensor_scalar accum_out (4x mode), one reciprocal
           per pair, e *= 1/rowsum
  * ACT is the bottleneck engine (two table passes over 8.4M elems at
    0.833 ns/elem ~ 110us); everything else hides under it.
  * No PE warmup: the first tile's matmuls ramp the p-state while the
    startup DMAs stream (adjacency pair 0 first, then A, qT chunk 0, kT in
    512-col chunks so the first matmul starts after ~2us).
  * Output DMAs are dispatched from the Pool sequencer so an out-DMA
    waiting on its data never blocks SP-issued input prefetches.
  * Output fp16 to HBM; host upcasts to f32.
"""
import numpy as np

import concourse.bacc as bacc
import concourse.mybir as mybir
from concourse.tile import TileContext
from concourse.bass_utils import run_bass_kernel_spmd

F32 = mybir.dt.float32
F16 = mybir.dt.float16
U8 = mybir.dt.uint8
AF = mybir.ActivationFunctionType
ALU = mybir.AluOpType

B, NQ, NK = 16, 2048, 2048
D = 256                 # KD = QD = KQ
CORES = 8
BPC = B // CORES        # batches per core
MT = 128                # query rows per tile
NMT = NQ // MT          # 16 m-tiles per batch
CH = 512                # psum bank free-dim (fp32)
NCH = NK // CH          # 4 n-chunks per scores row


def build(reps=1):
    nc = bacc.Bacc(None, target_bir_lowering=False)

    qT = nc.dram_tensor("qT", [BPC, D, NQ], F16, kind="ExternalInput")
    kT = nc.dram_tensor("kT", [BPC, D, NK], F16, kind="ExternalInput")
    adj = nc.dram_tensor("adj", [BPC, NQ, NK], U8, kind="ExternalInput")
    # A = Wq @ Wk^T, folded on host (weight preprocessing), laid out as
    # a_in[p, dc, e] = A[dc*128 + p, e]
    a_in = nc.dram_tensor("a_in", [D, D], F16, kind="ExternalInput")
    out = nc.dram_tensor("out", [BPC, NQ, NK], F16, kind="ExternalOutput")

    with TileContext(nc) as tc:
        with (
            tc.tile_pool(name="const", bufs=1) as cp,
            tc.tile_pool(name="mt", bufs=4) as mp,
            tc.tile_pool(name="pair", bufs=4) as pp,
            tc.tile_pool(name="ps", bufs=2, space="PSUM") as ps,
        ):
            batches = sorted(set(bb for _ in range(reps) for bb in range(BPC)))
            b0 = batches[0]

            tiles = [(b, mt) for _ in range(reps) for b in batches
                     for mt in range(NMT)]
            npairs = len(tiles) // 2

            # ---- pair prefetch: adjacency DMA + Pool uint8->fp16 convert ----
            pair_m16 = {}

            def prefetch_pair(p):
                pb, pmt = tiles[2 * p]
                adj_t = pp.tile([128, 2, NK], U8, name="adj_t")
                nc.sync.dma_start(
                    out=adj_t[:],
                    in_=adj[pb, pmt * MT:(pmt + 2) * MT, :].rearrange(
                        "(t p) n -> p t n", p=128))
                m16 = pp.tile([128, 2, NK], F16, name="m16")
                nc.gpsimd.tensor_copy(m16[:], adj_t[:])
                pair_m16[p] = m16

            # ---- startup: order DMAs by when the pipeline needs them ----
            # Dispatch cost dominates the startup (each DMA holds HWDGE
            # ~650ns), so the first few loads go out on THREE engine queues
            # in parallel: A on DVE, qT chunk 0 on ACT (idle until the first
            # tanh), kT chunks + adjacency on SP. kT streams in 512-col
            # chunks so the first score matmul starts after ~2us.
            a_t = cp.tile([128, 2, D], F16)    # a_t[p, dc, e] = A[dc*128+p, e]
            nc.gpsimd.dma_start(out=a_t[:], in_=a_in.rearrange("(c p) e -> p c e", p=128))
            qT_ts, kT_ts, qa_ts = {}, {}, {}
            for b in batches:
                qT_ts[b] = cp.tile([128, 2, NQ], F16, name=f"qT{b}")
                kT_ts[b] = cp.tile([128, 2, NK], F16, name=f"kT{b}")
                qa_ts[b] = cp.tile([128, 2, NQ], F16, name=f"qa{b}")
            nc.scalar.dma_start(
                out=qT_ts[b0][:, :, 0:CH],
                in_=qT[b0, :, 0:CH].rearrange("(c p) m -> p c m", p=128))
            for n in range(2):
                nc.sync.dma_start(
                    out=kT_ts[b0][:, :, n * CH:(n + 1) * CH],
                    in_=kT[b0, :, n * CH:(n + 1) * CH].rearrange(
                        "(c p) m -> p c m", p=128))
            prefetch_pair(0)
            for n in range(2, NCH):
                nc.sync.dma_start(
                    out=kT_ts[b0][:, :, n * CH:(n + 1) * CH],
                    in_=kT[b0, :, n * CH:(n + 1) * CH].rearrange(
                        "(c p) m -> p c m", p=128))
            prefetch_pair(1)

            ebias = cp.tile([128, 1], F32)
            nc.vector.memset(ebias[:], -20.0)

            def late_loads(i):
                if i == 1:
                    for dc in range(2):
                        nc.sync.dma_start(out=qT_ts[b0][:, dc, CH:],
                                          in_=qT[b0, dc * 128:(dc + 1) * 128, CH:])
                elif i in (3, 5, 7, 9) and len(batches) > 1:
                    # halves, so adjacency prefetches interleave between them
                    b1 = batches[1]
                    j = (i - 3) // 2
                    dst, src = ((qT_ts[b1], qT) if j < 2 else (kT_ts[b1], kT))
                    h = j % 2
                    nc.sync.dma_start(
                        out=dst[:, :, h * NK // 2:(h + 1) * NK // 2],
                        in_=src[b1, :, h * NK // 2:(h + 1) * NK // 2].rearrange(
                            "(c p) m -> p c m", p=128))

            def qa_chunk(b, dp, mc):
                qa_ps = ps.tile([128, CH], F32, tag="sc", name="qa_ps")
                for dc in range(2):
                    nc.tensor.matmul(
                        qa_ps[:],
                        a_t[:, dc, dp * 128:(dp + 1) * 128],
                        qT_ts[b][:, dc, mc * CH:(mc + 1) * CH],
                        start=(dc == 0),
                        stop=(dc == 1),
                    )
                nc.gpsimd.tensor_copy(
                    qa_ts[b][:, dp, mc * CH:(mc + 1) * CH], qa_ps[:])

            qa_jobs = [(b, dp, mc) for b in batches for mc in range(NCH)
                       for dp in range(2)]
            qa_chunk(*qa_jobs[0])
            qa_chunk(*qa_jobs[1])
            qa_left = qa_jobs[2:]

            # ---- software-pipelined m-tile loop, pair-granular epilogue ----
            # ACT order: tanh(2p), exp(pair p-1), tanh(2p+1), ... so the pair
            # exp (one 4096-wide op) fills ACT while DVE masks tile 2p and
            # the pair p-1 epilogue (rowsum/normalize) runs.
            pair_t = {}       # p -> tanh pair tile
            pair_e = {}       # p -> exp pair tile

            def pair_epilogue(p, split_dma=False):
                """rowsum + normalize + out DMA for pair p (deps all ready)."""
                pb, pmt = tiles[2 * p]
                t_pr = pair_t.pop(p)
                e_pr = pair_e.pop(p)
                rsum = mp.tile([128, 2], F32, bufs=2, name="rsum")
                rcp = mp.tile([128, 2], F32, bufs=2, name="rcp")
                for h in range(2):
                    nc.vector.tensor_scalar(
                        t_pr[:, h], e_pr[:, h], 1.0, 0.0,
                        op0=ALU.mult, op1=ALU.add, accum_out=rsum[:, h:h + 1])
                nc.vector.reciprocal(rcp[:], rsum[:])
                for h in range(2):
                    nc.vector.tensor_scalar_mul(e_pr[:, h], e_pr[:, h],
                                                rcp[:, h:h + 1])
                if split_dma:
                    for h in range(2):
                        nc.sync.dma_start(
                            out=out[pb, (pmt + h) * MT:(pmt + h + 1) * MT, :],
                            in_=e_pr[:, h])
                else:
                    nc.sync.dma_start(
                        out=out[pb, pmt * MT:(pmt + 2) * MT, :].rearrange(
                            "(t p) n -> p t n", p=128),
                        in_=e_pr[:])

            for idx, (b, mt) in enumerate(tiles):
                p, half = divmod(idx, 2)
                if half == 0:
                    pair_t[p] = mp.tile([128, 2, NK], F16, name="t_pr")
                if idx == 0:
                    # tile 0: per-chunk psum tiles ladder through the two
                    # PSUM buffers, so each tanh chunk starts as soon as its
                    # own 2 matmuls are done (tile-granular deps otherwise
                    # make the first tanh wait for all 8)
                    for n in range(NCH):
                        c_ps = ps.tile([128, CH], F32, tag="sc", name="sc_c")
                        for dp in range(2):
                            nc.tensor.matmul(
                                c_ps[:],
                                qa_ts[b][:, dp, mt * MT:(mt + 1) * MT],
                                kT_ts[b][:, dp, n * CH:(n + 1) * CH],
                                start=(dp == 0),
                                stop=(dp == 1),
                            )
                        nc.scalar.activation(
                            pair_t[p][:, half, n * CH:(n + 1) * CH],
                            c_ps[:], AF.Tanh, scale=1.0 / 16.0)
                else:
                    # scores matmuls, n-outer
                    sc_ps = ps.tile([128, NK], F32, tag="sc", name="sc_ps")
                    for n in range(NCH):
                        for dp in range(2):
                            nc.tensor.matmul(
                                sc_ps[:, n * CH:(n + 1) * CH],
                                qa_ts[b][:, dp, mt * MT:(mt + 1) * MT],
                                kT_ts[b][:, dp, n * CH:(n + 1) * CH],
                                start=(dp == 0),
                                stop=(dp == 1),
                            )
                    nc.scalar.activation(pair_t[p][:, half], sc_ps[:],
                                         AF.Tanh, scale=1.0 / 16.0)
                if qa_left and idx >= 1:
                    qa_chunk(*qa_left.pop(0))
                if half == 0 and p > 0:
                    # previous pair: one 4096-wide exp keeps ACT busy through
                    # this tile's DVE mask-add
                    e_pr = pp.tile([128, 2, NK], F16, name="e_pr")
                    nc.scalar.activation(e_pr[:], pair_t[p - 1][:], AF.Exp,
                                         scale=10.0, bias=ebias[:])
                    pair_e[p - 1] = e_pr
                # mask add (tensor_tensor runs in the 2x DVE mode)
                nc.vector.tensor_tensor(
                    pair_t[p][:, half], pair_t[p][:, half],
                    pair_m16[p][:, half], op=ALU.add)
                if half == 0 and p > 0:
                    pair_epilogue(p - 1)
                if idx == len(tiles) - 1:
                    # drain: last pair per tile so the tail is one tile's
                    # exp+norm+DMA instead of a whole pair's (the rest of the
                    # tail is out-DMA drain, which chunking can't shrink)
                    e_pr = pp.tile([128, 2, NK], F16, name="e_pr")
                    pair_e[p] = e_pr
                    t_pr = pair_t[p]
                    rsum = mp.tile([128, 2], F32, bufs=2, name="rsum")
                    rcp = mp.tile([128, 2], F32, bufs=2, name="rcp")
                    for h in range(2):
                        nc.scalar.activation(e_pr[:, h], t_pr[:, h], AF.Exp,
                                             scale=10.0, bias=ebias[:])
                        nc.vector.tensor_scalar(
                            t_pr[:, h], e_pr[:, h], 1.0, 0.0,
                            op0=ALU.mult, op1=ALU.add,
                            accum_out=rsum[:, h:h + 1])
                        nc.vector.reciprocal(rcp[:, h:h + 1], rsum[:, h:h + 1])
                        nc.vector.tensor_scalar_mul(e_pr[:, h], e_pr[:, h],
                                                    rcp[:, h:h + 1])
                        nc.sync.dma_start(
                            out=out[b, (mt - 1 + h) * MT:(mt + h) * MT, :],
                            in_=e_pr[:, h])
                    pair_t.pop(p)
                    pair_e.pop(p)
                if half == 1:
                    late_loads(idx)
                    if p + 2 < npairs:
                        prefetch_pair(p + 2)
    nc.compile()
    return nc


_NC = None


def _get_nc():
    global _NC
    if _NC is None:
        _NC = build()
    return _NC


def kernel(k_inputs, q_inputs, adjancy, Wk, Wq):
    k_inputs = np.asarray(k_inputs, dtype=np.float32)
    q_inputs = np.asarray(q_inputs, dtype=np.float32)
    adjancy = np.asarray(adjancy, dtype=np.int32)
    Wk = np.asarray(Wk, dtype=np.float32)
    Wq = np.asarray(Wq, dtype=np.float32)
    nc = _get_nc()
    a_in = (Wq @ Wk.T).astype(np.float16)
    in_maps = []
    for c in range(CORES):
        lo, hi = c * BPC, (c + 1) * BPC
        in_maps.append({
            "qT": np.ascontiguousarray(
                q_inputs[lo:hi].transpose(0, 2, 1)).astype(np.float16),
            "kT": np.ascontiguousarray(
                k_inputs[lo:hi].transpose(0, 2, 1)).astype(np.float16),
            "adj": (adjancy[lo:hi] * 2).astype(np.uint8),
            "a_in": a_in,
        })
    res = run_bass_kernel_spmd(nc, in_maps, core_ids=list(range(CORES)))
    return np.concatenate(
        [res.results[c]["out"] for c in range(CORES)], axis=0
    ).astype(np.float32)
